# revision 1
# baseline (speedup 1.0000x reference)
"""Trainium2 Bass kernel for nn_BayerUpsample4x4.

The reference op: x [4,1,1024,1024] -> 16-channel polyphase 4x bilinear
(tent-filter) upsample, output [4,16,1024,1024].  Each output channel
k=(r,c) is x subsampled at rows≡r, cols≡c (mod 4), zero-upsampled x4 and
convolved with the separable 7x7 tent kernel == bilinear interpolation
with zero padding at image borders.

Kernel plan (per core; 8 cores = 4 batches x 2 row-halves):
  - vertical interpolation on TensorE:  Vt = V_rb.T @ X_rq  (banded fp32
    interpolation matrices, built host-side from `weight`)
  - PSUM evacuation + prescaling on ScalarE/VectorE: Vt, 0.25*Vt, 0.5*Vt,
    0.75*Vt   (scaled copies let the horizontal pass be one fused op per
    column phase)
  - horizontal interpolation on VectorE scalar_tensor_tensor:
        out[:, j0::4] = (Vt_lo * a_e) + (b_e*Vt)_hi
    with 4-column zero pads so borders need no special ops
  - e=0 column copies on GpSimd, final stores are dense 512KB DMAs
"""

import sys
for _p in ("/opt/trn_rl_repo", "/opt/pypackages"):
    if _p not in sys.path:
        sys.path.append(_p)

from contextlib import ExitStack

import numpy as np

import concourse.bass as bass
import concourse.tile as tile
from concourse import bacc, mybir
from concourse.bass_utils import run_bass_kernel_spmd

F32 = mybir.dt.float32
AF = mybir.ActivationFunctionType
OP = mybir.AluOpType

N_CORES = 8
H, W = 1024, 1024
HALF = 512               # output rows per core
SLAB = 768               # padded input slab rows per core

# (row, col) offset within each 4x4 block for channel k (matches reference)
OFFSETS = [(0, 0), (0, 2), (2, 0), (2, 2),
           (0, 1), (0, 3), (2, 1), (2, 3),
           (1, 0), (1, 2), (3, 0), (3, 2),
           (1, 1), (1, 3), (3, 1), (3, 3)]
K_OF = {rc: k for k, rc in enumerate(OFFSETS)}


def _emit(tc, xs, vm, out, kh):
    """Trace the per-core program.

    xs:  [768, 1024] f32 zero-padded input slab (rows h0-4 .. h0+763)
    vm:  [8, 128, 128] f32 vertical interp matrices, index r*2+b, [p, m]
    out: [16, 512, 1024] f32
    kh:  length-7 horizontal filter profile (numpy)
    """
    nc = tc.nc
    a_e = {e: float(kh[3 - e]) for e in (1, 2, 3)}   # weight of lo sample
    b_e = {e: float(kh[7 - e]) for e in (1, 2, 3)}   # weight of hi sample

    with ExitStack() as ctx:
        vpool = ctx.enter_context(tc.tile_pool(name="vmp", bufs=1))
        xpool = ctx.enter_context(tc.tile_pool(name="xp", bufs=3))
        pspool = ctx.enter_context(tc.tile_pool(name="psp", bufs=4, space="PSUM"))
        vtpool = ctx.enter_context(tc.tile_pool(name="vtp", bufs=2))
        opool = ctx.enter_context(tc.tile_pool(name="op", bufs=8))

        # ---- load all 8 V matrices into one [128, 8*128] tile ----
        vmt = vpool.tile([128, 8 * 128], F32, tag="vmt")
        nc.sync.dma_start(vmt[:], vm.rearrange("i p m -> p i m"))

        xs_rows = xs.rearrange("(t s) w -> s t w", s=4)   # [4, 192, 1024]

        for q in range(2):
            for r in range(4):
                xt = xpool.tile([128, W], F32, tag="xt")
                nc.sync.dma_start(xt[:], xs_rows[r][64 * q: 64 * q + 128, :])

                for b in range(2):
                    lhsT = vmt[:, (r * 2 + b) * 128: (r * 2 + b + 1) * 128]

                    # vt has 4 left pad cols; p25/p50/p75 have 4 right pad
                    vt = vtpool.tile([128, W + 4], F32, tag="vt")
                    p25 = vtpool.tile([128, W + 4], F32, tag="p25")
                    p50 = vtpool.tile([128, W + 4], F32, tag="p50")
                    p75 = vtpool.tile([128, W + 4], F32, tag="p75")
                    nc.vector.memset(vt[:, 0:4], 0.0)
                    nc.vector.memset(p25[:, W:W + 4], 0.0)
                    nc.vector.memset(p50[:, W:W + 4], 0.0)
                    nc.vector.memset(p75[:, W:W + 4], 0.0)

                    for ch in range(2):
                        ps = pspool.tile([128, 512], F32, tag="ps")
                        nc.tensor.matmul(
                            ps[:], lhsT=lhsT,
                            rhs=xt[:, 512 * ch: 512 * ch + 512],
                            start=True, stop=True,
                        )
                        sl = slice(4 + 512 * ch, 4 + 512 * ch + 512)
                        dl = slice(512 * ch, 512 * ch + 512)
                        # evac + prescale (ACT reads PSUM fast; P25 from SBUF on DVE at 2x)
                        nc.scalar.activation(vt[:, sl], ps[:], AF.Copy)
                        nc.scalar.activation(p50[:, dl], ps[:], AF.Copy,
                                             scale=b_e[2])
                        nc.scalar.activation(p75[:, dl], ps[:], AF.Copy,
                                             scale=b_e[3])
                        nc.vector.tensor_scalar_mul(p25[:, dl], vt[:, sl],
                                                    b_e[1])

                    # grouped [128, 257, 4] views for phase-strided access
                    vtv = vt.rearrange("p (u s) -> p u s", s=4)
                    pv = {1: p25.rearrange("p (u s) -> p u s", s=4),
                          2: p50.rearrange("p (u s) -> p u s", s=4),
                          3: p75.rearrange("p (u s) -> p u s", s=4)}

                    for c in range(4):
                        k = K_OF[(r, c)]
                        oc = opool.tile([128, W], F32, tag="oc")
                        ov = oc.rearrange("p (u s) -> p u s", s=4)
                        # e = 0: plain copy of the vertical result
                        nc.gpsimd.tensor_copy(ov[:, :, c], vtv[:, 1:257, c])
                        for e in (1, 2, 3):
                            j0 = (c + e) % 4
                            st = 4 + j0 - e          # start col (1..6)
                            u0, s0 = divmod(st, 4)
                            lo = vtv[:, u0:u0 + 256, s0]
                            hi = pv[e][:, u0:u0 + 256, s0]
                            nc.vector.scalar_tensor_tensor(
                                ov[:, :, j0], lo, a_e[e], hi,
                                op0=OP.mult, op1=OP.add)
                        row0 = 256 * q + 128 * b
                        nc.sync.dma_start(out[k, row0:row0 + 128, :], oc[:])


_CACHE = {}


def _build_module(kh):
    key = tuple(np.asarray(kh, np.float64).tolist())
    if key in _CACHE:
        return _CACHE[key]
    nc = bacc.Bacc("TRN2", target_bir_lowering=False, debug=False)
    xs = nc.dram_tensor("xs", [SLAB, W], F32, kind="ExternalInput").ap()
    vm = nc.dram_tensor("vm", [8, 128, 128], F32, kind="ExternalInput").ap()
    out = nc.dram_tensor("out", [16, HALF, W], F32, kind="ExternalOutput").ap()
    with tile.TileContext(nc) as tc:
        _emit(tc, xs, vm, out, kh)
    nc.compile()
    _CACHE[key] = nc
    return nc


def _vmats(kv):
    V = np.zeros((8, 128, 128), np.float32)
    for r in range(4):
        for b in range(2):
            for m in range(128):
                d = (m - r) % 4
                p_lo = 32 * b + (m - r - d) // 4 + 1
                V[r * 2 + b, p_lo, m] += kv[3 - d]
                if d > 0:
                    V[r * 2 + b, p_lo + 1, m] += kv[7 - d]
    return V


def _slabs(x):
    s = np.zeros((N_CORES, SLAB, W), np.float32)
    for core in range(N_CORES):
        n, half = divmod(core, 2)
        g0 = 512 * half - 4
        s0, s1 = max(0, g0), min(H, g0 + SLAB)
        s[core, s0 - g0: s1 - g0] = x[n, 0, s0:s1]
    return s


def kernel(x, weight):
    x = np.asarray(x, np.float32)
    weight = np.asarray(weight, np.float32)
    assert x.shape == (4, 1, H, W), x.shape
    k2 = weight[0, 0]
    kv = k2[:, 3].astype(np.float64)   # vertical profile (k1)
    kh = k2[3, :].astype(np.float64)   # horizontal profile (k1)

    nc = _build_module(kh)
    V = _vmats(kv)
    slabs = _slabs(x)
    in_maps = [{"xs": slabs[c], "vm": V} for c in range(N_CORES)]
    res = run_bass_kernel_spmd(nc, in_maps, list(range(N_CORES)))

    full = np.empty((4, 16, H, W), np.float32)
    for core in range(N_CORES):
        n, half = divmod(core, 2)
        full[n, :, 512 * half: 512 * half + 512, :] = res.results[core]["out"]
    return full
